# revision 17
# baseline (speedup 1.0000x reference)
"""Causal multi-head attention on 8 trn2 NeuronCores.

Problem (hardcoded): x[4, 2048, 768], w_attn[768, 2304], b_attn[2304],
w_proj[768, 768], b_proj[768]; H=6 heads, D=128 head dim; fp32.

Sharding: core c = 2*b + g handles batch b and head-group g (heads
3g..3g+2).  Each core computes Q/K/V projections for its 3 heads over the
full sequence, full causal attention for those heads, and a PARTIAL output
projection (w_proj rows of its heads).  The host sums the two partials per
batch and adds the bias terms.  No cross-core communication.

Bias algebra (host/device split):
  - b_q is added on device (affects scores).
  - b_k is dropped entirely: it shifts every score in a softmax row by the
    same constant, which cancels.
  - b_v is dropped on device: softmax rows sum to 1, so attn @ (v + b_v)
    = attn @ v + b_v; the constant (b_v @ w_proj + b_proj) is added on host.

Device layouts (all transposed, so no on-chip transposes are needed):
  - x is fed as xT [C=768, S=2048];  Q^T/K^T come out as [D, S] per head.
  - scores are computed transposed: sT[kv, rows] = K @ Q^T  (kv on PSUM
    partitions), masked causally, exp'd on the scalar engine straight into
    SBUF.  attn_outT[D, rows] = sum_j V_j^T(lhsT) @ expS_j; the softmax
    denominators come from an all-ones-lhsT matmul accumulated alongside
    (which also replicates them across partitions for the elementwise
    divide).
  - output is written transposed ([768, 2048] partial); host transposes.

Matmuls run as float32r (full fp32 data, reduced-precision PE mode, 1
cycle/row at free-dim >= 256 vs 4 for fp32).  Causal structure is rounded
to 512-row query groups: group t (rows 512t..512t+511) attends to kv
[0, 512(t+1)); the diagonal 4 kv chunks get a host-supplied -1e9 additive
mask.

Scheduling: inputs are split into several DMAs ordered by first use so PE
starts early instead of waiting for the full ~11MB; the attention inner
loop is emitted with a one-batch software-pipeline skew (PE is in-order,
so the PV/rowsum matmuls that wait on exp(batch i) are emitted after the
score matmuls of batch i+1, hiding the scalar-engine latency).
"""

import math
from contextlib import ExitStack

import numpy as np

import concourse.bacc as bacc
import concourse.bass as bass
import concourse.mybir as mybir
import concourse.tile as tile
from concourse import bass_utils

B, S, C = 4, 2048, 768
H, D = 6, 128
HL = 3          # heads per core
CK = C // 128   # 6 contraction chunks
R = 512         # query rows per group
G = S // R      # 4 groups
N_CORES = 8
F32 = mybir.dt.float32
F32R = mybir.dt.float32r
MASK_VAL = -1e9
INV_SQRT_D = 1.0 / math.sqrt(D)
AUXW = HL + 4 * R  # bq columns + 4 mask tiles


def _emit(ctx: ExitStack, tc: tile.TileContext, xa, wav, waqk, aux, ones, wp, outT):
    nc = tc.nc

    singles = ctx.enter_context(tc.tile_pool(name="singles", bufs=1))
    expool = ctx.enter_context(tc.tile_pool(name="expool", bufs=2))
    aopool = ctx.enter_context(tc.tile_pool(name="aopool", bufs=2))
    otpool = ctx.enter_context(tc.tile_pool(name="otpool", bufs=2))
    rspool = ctx.enter_context(tc.tile_pool(name="rspool", bufs=2))
    psum = ctx.enter_context(tc.tile_pool(name="psum", bufs=2, space="PSUM"))

    # ---- resident loads, split + ordered by first use ----
    xa_sb = singles.tile([128, G, CK, R], F32R)     # x, token-chunk major
    wav_sb = singles.tile([128, CK, HL * D], F32R)  # v columns of w_attn
    waqk_sb = singles.tile([128, CK, 2 * HL * D], F32R)
    aux_sb = singles.tile([128, AUXW], F32)         # [bq | 4 causal masks]
    ones_sb = singles.tile([128, 128], F32R)
    wp_sb = singles.tile([128, HL, C], F32R)

    # Ordered by first use: V-projection work (xa chunk 0 + v weights) is the
    # shortest critical prefix, so PE starts ~8us in.
    nc.sync.dma_start(xa_sb[:, 0], xa[:, :CK * R].rearrange("p (c s) -> p c s", c=CK))
    nc.sync.dma_start(wav_sb, wav.rearrange("p (c n) -> p c n", c=CK))
    nc.sync.dma_start(waqk_sb, waqk.rearrange("p (c n) -> p c n", c=CK))
    nc.sync.dma_start(aux_sb, aux)
    nc.sync.dma_start(ones_sb, ones)
    for n in range(1, G):
        nc.sync.dma_start(
            xa_sb[:, n],
            xa[:, n * CK * R:(n + 1) * CK * R].rearrange("p (c s) -> p c s", c=CK))
    nc.sync.dma_start(wp_sb, wp.rearrange("p (f n) -> p f n", f=HL))

    bq_sb = aux_sb[:, 0:HL]

    def mask_ap(k):
        return aux_sb[:, HL + k * R: HL + (k + 1) * R]

    # ---- QKV projections, interleaved per x token-chunk so PE work tracks
    # DMA arrival (xa0, wav, waqk, xa1, xa2, xa3).  V for chunk n needs only
    # xa chunk n + v weights (shortest critical prefix starts PE earliest).
    # V is in [token, feature] layout: V_sb[:, r, :] = rows 128r..128r+127.
    V_sb = singles.tile([128, S // 128, HL * D], F32R)
    qkT_sb = singles.tile([128, 2 * HL, S], F32R)
    for n in range(G):
        for r in range(4 * n, 4 * n + 4):
            ps = psum.tile([128, R], F32, tag="st")
            for c in range(CK):
                nc.tensor.matmul(
                    ps[:, :HL * D],
                    lhsT=xa_sb[:, n, c, (r % 4) * 128:(r % 4 + 1) * 128],
                    rhs=wav_sb[:, c, :],
                    start=(c == 0),
                    stop=(c == CK - 1),
                )
            nc.vector.tensor_copy(V_sb[:, r, :], ps[:, :HL * D])
        for f in range(2 * HL):
            ps = psum.tile([128, R], F32, tag="st")
            for c in range(CK):
                nc.tensor.matmul(
                    ps,
                    lhsT=waqk_sb[:, c, f * 128:(f + 1) * 128],
                    rhs=xa_sb[:, n, c, :],
                    start=(c == 0),
                    stop=(c == CK - 1),
                )
            if f < HL:
                nc.scalar.add(qkT_sb[:, f, n * R:(n + 1) * R], ps, bq_sb[:, f:f + 1])
            else:
                nc.vector.tensor_copy(qkT_sb[:, f, n * R:(n + 1) * R], ps)

    # ---- attention + output projection, software-pipelined ----
    # Emission order == PE execution order (in-order engine).  Defer each
    # batch's PV/rowsum matmuls (which wait on its exp) by TWO score batches,
    # across head/group boundaries, so exp latency and the
    # recip/norm/proj chain never stall PE.
    pending = []
    proj_queue = []

    def push(fn):
        pending.append(fn)
        while len(pending) > 2:
            pending.pop(0)()

    def pop_proj(k):
        for _ in range(min(k, len(proj_queue))):
            proj_queue.pop(0)()

    def drain():
        while pending:
            pending.pop(0)()
        while proj_queue:
            proj_queue.pop(0)()

    for t in range(G):
        rows = slice(t * R, (t + 1) * R)
        nk = 4 * (t + 1)
        ao = aopool.tile([128, HL, R], F32R, tag="ao")
        for h in range(HL):
            pv = psum.tile([128, R], F32, tag="pv")
            rs = psum.tile([128, R], F32, tag="rs")
            for jb in range(nk // 2):
                if jb == 1:
                    pop_proj(2)  # head-start filler hides exp/norm latency
                st = psum.tile([128, 2, R], F32, tag="st")
                for u in range(2):
                    j = 2 * jb + u
                    nc.tensor.matmul(
                        st[:, u, :],
                        lhsT=qkT_sb[:, HL + h, j * 128:(j + 1) * 128],
                        rhs=qkT_sb[:, h, rows],
                        start=True,
                        stop=True,
                    )
                    if j >= nk - 4:
                        nc.vector.tensor_tensor(
                            st[:, u, :], st[:, u, :],
                            mask_ap(j - (nk - 4)), mybir.AluOpType.add,
                        )
                ex = expool.tile([128, 2, R], F32R, tag="ex")
                nc.scalar.activation(
                    ex, st, mybir.ActivationFunctionType.Exp, scale=INV_SQRT_D,
                )

                def consume(jb=jb, h=h, t=t, pv=pv, rs=rs, ex=ex, ao=ao, nk=nk):
                    for u in range(2):
                        j = 2 * jb + u
                        nc.tensor.matmul(
                            pv,
                            lhsT=V_sb[:, j, h * D:(h + 1) * D],
                            rhs=ex[:, u, :],
                            start=(j == 0),
                            stop=(j == nk - 1),
                        )
                        nc.tensor.matmul(
                            rs,
                            lhsT=ones_sb,
                            rhs=ex[:, u, :],
                            start=(j == 0),
                            stop=(j == nk - 1),
                        )
                    if jb == nk // 2 - 1:
                        rsr = rspool.tile([128, R], F32, tag="rsr")
                        nc.vector.reciprocal(rsr, rs)
                        nc.vector.tensor_tensor(
                            ao[:, h, :], pv, rsr, mybir.AluOpType.mult)
                        if h == HL - 1:
                            proj_queue.extend(
                                _proj_obs(nc, psum, otpool, wp_sb, ao, outT, t))

                push(consume)
    drain()


def _proj_obs(nc, psum, otpool, wp_sb, ao, outT, t):
    rows = slice(t * R, (t + 1) * R)

    def one(ob):
        ps = psum.tile([128, R], F32, tag="pv")
        for fc in range(HL):
            nc.tensor.matmul(
                ps,
                lhsT=wp_sb[:, fc, ob * 128:(ob + 1) * 128],
                rhs=ao[:, fc, :],
                start=(fc == 0),
                stop=(fc == HL - 1),
            )
        ot = otpool.tile([128, R], F32, tag="ot")
        if ob % 2 == 0:
            nc.scalar.copy(ot, ps)
        else:
            nc.vector.tensor_copy(ot, ps)
        nc.sync.dma_start(outT[ob * 128:(ob + 1) * 128, rows], ot)

    return [lambda ob=ob: one(ob) for ob in range(C // 128)]


_CACHED = None


def _build():
    global _CACHED
    if _CACHED is not None:
        return _CACHED
    nc = bacc.Bacc(
        "TRN2",
        target_bir_lowering=False,
        debug=False,
        enable_asserts=False,
        num_devices=N_CORES,
    )
    xa = nc.dram_tensor("xa", [128, G * CK * R], F32R, kind="ExternalInput").ap()
    wav = nc.dram_tensor("wav", [128, CK * HL * D], F32R, kind="ExternalInput").ap()
    waqk = nc.dram_tensor("waqk", [128, CK * 2 * HL * D], F32R, kind="ExternalInput").ap()
    aux = nc.dram_tensor("aux", [128, AUXW], F32, kind="ExternalInput").ap()
    ones = nc.dram_tensor("ones", [128, 128], F32R, kind="ExternalInput").ap()
    wp = nc.dram_tensor("wp", [128, HL * C], F32R, kind="ExternalInput").ap()
    outT = nc.dram_tensor("outT", [C, S], F32, kind="ExternalOutput").ap()
    with tile.TileContext(nc) as tc, ExitStack() as ctx:
        _emit(ctx, tc, xa, wav, waqk, aux, ones, wp, outT)
    nc.compile()
    _CACHED = nc
    return nc


def _pmajor(a2d):
    """[n*128, w] -> [128, n*w]: partition-major shuffle for one-DMA loads."""
    n = a2d.shape[0] // 128
    w = a2d.shape[1]
    return np.ascontiguousarray(
        a2d.reshape(n, 128, w).transpose(1, 0, 2).reshape(128, n * w))


def _masks():
    i = np.arange(R)[None, :]
    j = np.arange(128)[:, None]
    cols = [np.where(i >= j + 128 * k, 0.0, MASK_VAL).astype(np.float32)
            for k in range(4)]
    return np.concatenate(cols, axis=1)  # [128, 4*R]


def shard_inputs(x, w_attn, b_attn, w_proj):
    """Per-core input dicts for cores 0..7 (core = 2*batch + head_group)."""
    masks = _masks()
    ones = np.ones((128, 128), np.float32)
    in_maps = []
    for c in range(N_CORES):
        b, g = divmod(c, 2)
        lo, hi = g * HL * D, (g + 1) * HL * D
        wav = w_attn[:, 2 * C + lo:2 * C + hi]
        waqk = np.concatenate(
            [w_attn[:, lo:hi], w_attn[:, C + lo:C + hi]], axis=1)
        xT = np.ascontiguousarray(x[b].T)  # [768, 2048]
        xa = np.concatenate(
            [_pmajor(xT[:, n * R:(n + 1) * R]) for n in range(G)], axis=1)
        bq = np.ascontiguousarray(b_attn[lo:hi]).reshape(HL, 128).T  # [128, HL]
        in_maps.append({
            "xa": xa,
            "wav": _pmajor(wav),
            "waqk": _pmajor(waqk),
            "aux": np.concatenate([bq, masks], axis=1),
            "ones": ones,
            "wp": _pmajor(w_proj[lo:hi, :]),
        })
    return in_maps


def combine_outputs(parts, b_attn, w_proj, b_proj):
    """parts[c] = outT partial [768, 2048] from core c."""
    bias = b_attn[2 * C:].astype(np.float64) @ w_proj.astype(np.float64) + b_proj
    out = np.empty((B, S, C), np.float32)
    for b in range(B):
        acc = parts[2 * b].astype(np.float32) + parts[2 * b + 1]
        out[b] = acc.T + bias.astype(np.float32)[None, :]
    return out


def kernel(x, w_attn, b_attn, w_proj, b_proj, **run_kwargs):
    x = np.asarray(x, np.float32)
    w_attn = np.asarray(w_attn, np.float32)
    b_attn = np.asarray(b_attn, np.float32)
    w_proj = np.asarray(w_proj, np.float32)
    b_proj = np.asarray(b_proj, np.float32)

    nc = _build()
    in_maps = shard_inputs(x, w_attn, b_attn, w_proj)
    res = bass_utils.run_bass_kernel_spmd(
        nc, in_maps, core_ids=list(range(N_CORES)), **run_kwargs
    )
    parts = [r["outT"] for r in res.results]
    out = combine_outputs(parts, b_attn, w_proj, b_proj)
    kernel.last_results = res
    return out
